# revision 10
# baseline (speedup 1.0000x reference)
"""Block-diagonal GRU cell on 8 TRN2 NeuronCores — one block per core.

Math per block n (torch GRUCell):
  gi = x_n @ W_ih[n].T + b_ih[n]        (B, 3*BS)
  gh = h_n @ W_hh[n].T + b_hh[n]
  r = sigmoid(gi_r + gh_r); z = sigmoid(gi_z + gh_z)
  ng = tanh(gi_n + r * gh_n)
  h' = ng + z * (h_n - ng)

On-chip layout (per core): everything transposed so the contraction
(feature) dim is the SBUF partition dim and gates land on PSUM
partitions — biases then apply as ACT per-partition bias operands.
  A  = [W_ih[n].T ; W_hh[n].T]  -> (1024 feat, 1536 gates), blocked per
       128-gate column group so each group's DMA is contiguous.
  U  = [x_n.T ; h_n.T]          -> (1024 feat, 1024 batch)
  out = h'.T                    -> (512, 1024), un-transposed on host.
r/z gates accumulate x- and h-matmuls into one PSUM bank (8 k-steps);
the n gate keeps i_n / h_n in separate banks. Matmuls run as float32r
(full-rate fp32 PE mode; moving free dim 512 >= 256).
"""

import os
import sys

import numpy as np

try:
    import concourse.bass as bass
except ImportError:  # fresh grading dir: fall back to the repo checkout
    sys.path.insert(0, "/opt/trn_rl_repo")
    import concourse.bass as bass

import concourse.mybir as mybir
import concourse.tile as tile
from concourse import bacc
from concourse.bass import ts
from concourse.bass_utils import run_bass_kernel_spmd

B = 1024            # batch
NB = 8              # blocks == cores
BS = 512            # hidden block size
G3 = 3 * BS         # gates per block (r, z, n)
KF = 1024           # contraction feats per core: 512 input + 512 hidden
P = 128
KT = KF // P        # 8 k-tiles
GT = G3 // P        # 12 gate column groups: 0-3 r, 4-7 z, 8-11 n
NBC = 2             # batch chunks
BC = B // NBC       # 512 (one PSUM bank of fp32)

F32 = mybir.dt.float32
F32R = mybir.dt.float32r
AFT = mybir.ActivationFunctionType

_cache: dict = {}
LAST_RESULTS = None  # BassKernelResults of the most recent run (for test.py)


def _build_nc():
    nc = bacc.Bacc("TRN2", target_bir_lowering=False, debug=False, num_devices=NB)
    a_d = nc.dram_tensor("a", [GT, P, KT, P], F32R, kind="ExternalInput").ap()
    u_d = nc.dram_tensor("u", [KF, B], F32R, kind="ExternalInput").ap()
    brz_d = nc.dram_tensor("brz", [P, 8], F32, kind="ExternalInput").ap()
    bn_d = nc.dram_tensor("bn", [P, 8], F32, kind="ExternalInput").ap()
    o_d = nc.dram_tensor("o", [BS, B], F32, kind="ExternalOutput").ap()

    with tile.TileContext(nc) as tc:
        with (
            tc.tile_pool(name="persist", bufs=1) as persist,
            tc.tile_pool(name="tmp", bufs=3) as tmp,
            tc.tile_pool(name="outp", bufs=4) as outp,
            tc.tile_pool(name="psum", bufs=8, space="PSUM") as psum,
        ):
            brz_sb = persist.tile([P, 8], F32, name="brz_sb")
            nc.sync.dma_start(brz_sb[:], brz_d[:])
            bn_sb = persist.tile([P, 8], F32, name="bn_sb")
            nc.sync.dma_start(bn_sb[:], bn_d[:])

            # U k-tiles [128 feat, B]; first batch-half first so compute can
            # start before the second half lands.
            U = [persist.tile([P, B], F32R, name=f"u{k}") for k in range(KT)]
            for k in range(KT):
                nc.sync.dma_start(U[k][:, ts(0, BC)], u_d[ts(k, P), ts(0, BC)])
            # A gate-column groups [128 feat-in-ktile, KT, 128 gate cols]
            A = [persist.tile([P, KT, P], F32R, name=f"a{g}") for g in range(GT)]
            for g in range(GT):
                nc.sync.dma_start(A[g][:], a_d[g])
            for k in range(KT):
                nc.sync.dma_start(U[k][:, ts(1, BC)], u_d[ts(k, P), ts(1, BC)])

            # r (g 0-3) and z (g 4-7) gate values, persistent per batch pass
            rz = [persist.tile([P, B], F32, name=f"rz{g}") for g in range(8)]

            for bc in range(NBC):
                for g in range(8):  # r and z: x- and h-GEMMs fused in PSUM
                    ps = psum.tile([P, BC], F32, name="ps", tag="ps")
                    for k in range(KT):
                        nc.tensor.matmul(
                            ps[:],
                            A[g][:, k, :],
                            U[k][:, ts(bc, BC)],
                            start=(k == 0),
                            stop=(k == KT - 1),
                        )
                    nc.scalar.activation(
                        rz[g][:, ts(bc, BC)], ps[:], AFT.Sigmoid,
                        bias=brz_sb[:, g : g + 1],
                    )
                for j in range(4):  # n gate + combine, rows f = j*128..
                    g = 8 + j
                    ps_i = psum.tile([P, BC], F32, name="ps_i", tag="ps")
                    for k in range(4):
                        nc.tensor.matmul(
                            ps_i[:],
                            A[g][:, k, :],
                            U[k][:, ts(bc, BC)],
                            start=(k == 0),
                            stop=(k == 3),
                        )
                    ps_h = psum.tile([P, BC], F32, name="ps_h", tag="ps")
                    for k in range(4, KT):
                        nc.tensor.matmul(
                            ps_h[:],
                            A[g][:, k, :],
                            U[k][:, ts(bc, BC)],
                            start=(k == 4),
                            stop=(k == KT - 1),
                        )
                    hnb = tmp.tile([P, BC], F32, name="hnb")
                    nc.scalar.activation(
                        hnb[:], ps_h[:], AFT.Identity, bias=bn_sb[:, 4 + j : 5 + j]
                    )
                    t = tmp.tile([P, BC], F32, name="t")
                    nc.vector.tensor_mul(t[:], rz[j][:, ts(bc, BC)], hnb[:])
                    t2 = tmp.tile([P, BC], F32, name="t2")
                    nc.vector.tensor_add(t2[:], t[:], ps_i[:])
                    nt = tmp.tile([P, BC], F32, name="nt")
                    nc.scalar.activation(
                        nt[:], t2[:], AFT.Tanh, bias=bn_sb[:, j : j + 1]
                    )
                    d = tmp.tile([P, BC], F32, name="d")
                    nc.vector.tensor_sub(
                        d[:], U[4 + j][:, ts(bc, BC)].bitcast(F32), nt[:]
                    )
                    e = tmp.tile([P, BC], F32, name="e")
                    nc.vector.tensor_mul(e[:], rz[4 + j][:, ts(bc, BC)], d[:])
                    o_t = outp.tile([P, BC], F32, name="o_t")
                    nc.vector.tensor_add(o_t[:], nt[:], e[:])
                    nc.sync.dma_start(o_d[ts(j, P), ts(bc, BC)], o_t[:])

    nc.compile()
    return nc


def _round_fp32r(a):
    """Round fp32 to the fp32r grid (E8M11: low 12 mantissa bits zero, RNE)."""
    b = np.ascontiguousarray(a, dtype=np.float32).view(np.uint32)
    lsb = (b >> 12) & 1
    out = ((b + 0x7FF + lsb) & np.uint32(0xFFFFF000)).view(np.float32)
    return out


def _prep_core_inputs(x, h, W_ih, W_hh, b_ih, b_hh, n):
    a_full = np.concatenate([W_ih[n].T, W_hh[n].T], axis=0)       # (1024, 1536)
    a_re = _round_fp32r(
        a_full.reshape(KT, P, GT, P).transpose(2, 1, 0, 3)
    )                                                             # (GT, P, KT, P)
    u = _round_fp32r(
        np.concatenate(
            [x[:, n * BS : (n + 1) * BS].T, h[:, n * BS : (n + 1) * BS].T], axis=0
        )
    )                                                             # (1024, 1024)
    brz = np.ascontiguousarray((b_ih[n, : 2 * BS] + b_hh[n, : 2 * BS]).reshape(8, P).T)
    bn = np.ascontiguousarray(
        np.concatenate(
            [b_ih[n, 2 * BS :].reshape(4, P).T, b_hh[n, 2 * BS :].reshape(4, P).T],
            axis=1,
        )
    )                                                             # (P, 8)
    return {"a": a_re, "u": u, "brz": brz, "bn": bn}


def kernel(x, h, W_ih, W_hh, b_ih, b_hh):
    global LAST_RESULTS
    x = np.asarray(x, dtype=np.float32)
    h = np.asarray(h, dtype=np.float32)
    W_ih = np.asarray(W_ih, dtype=np.float32)
    W_hh = np.asarray(W_hh, dtype=np.float32)
    b_ih = np.asarray(b_ih, dtype=np.float32)
    b_hh = np.asarray(b_hh, dtype=np.float32)

    if "nc" not in _cache:
        _cache["nc"] = _build_nc()
    nc = _cache["nc"]

    in_maps = [
        _prep_core_inputs(x, h, W_ih, W_hh, b_ih, b_hh, n) for n in range(NB)
    ]
    trace = os.environ.get("BASS_KERNEL_TRACE") == "1"
    res = run_bass_kernel_spmd(nc, in_maps, list(range(NB)), trace=trace)
    LAST_RESULTS = res
    return np.concatenate([res.results[n]["o"].T for n in range(NB)], axis=1)
